# revision 6
# baseline (speedup 1.0000x reference)
"""NT-Xent / SimCLR contrastive loss on 8 Trainium2 NeuronCores.

Math (matches the jax reference):
    z = l2_normalize(concat([emb_i, emb_j]))          # [2B, D] unit rows
    sim = z @ z.T                                     # cosine similarities
    denom_r = sum_{j != r} exp(sim_rj / T)
    pos_r   = z_r . z_{(r+B) mod 2B}                  # the positive pair
    loss = mean_r( log(denom_r) - pos_r / T )

v3 — symmetric-circulant sharding + fp8 DoubleRow matmuls:
  sim is symmetric, so only the ~33.5M unique entries are exp'd (the
  exp on the ACT engine is the hard bottleneck at 1 elem/lane/cycle).
  The 8192 rows form 16 blocks of 512; core c owns row-blocks r0=2c and
  r1=2c+1 and computes blocks (r0, r0+t mod 16) for t=0..8 and
  (r1, r1+t mod 16) for t=0..7 — 17 of 32 blocks, balanced, and each
  unordered block pair appears exactly once globally.  Row sums come
  free from the ACT accumulator; the transpose contribution of each
  off-diagonal block is a column sum computed with a ones-vector
  matmul on the PE (bf16 exp values, fp32 accumulate).

  Per-core pipeline:
    - load a [5120, 256] fp32 panel (10 consecutive row blocks, the
      union of everything the core touches)
    - sum-of-squares + rsqrt via Ln/Exp (one ACT table set), normalize
      on the DVE, transpose via PE identity matmuls, cast to fp8e4 on
      the PSUM->SBUF copy
    - 68 fp8 DoubleRow matmuls (K=256 in a single pass) into
      [128, <=1536] PSUM groups; ACT computes exp(2*sim) straight out
      of PSUM with fused row accumulation, bf16 exp values to SBUF
    - 60 ones-matmuls produce the 15 off-diagonal column sums
    - positives as exact fp32 row dots of the raw panel (host applies
      the inverse norms)
  Host assembles denom[8192] from row/col partials, subtracts the e^2
  self term, and takes mean(log(denom) - 2*pos) in float64.
"""

import numpy as np
from contextlib import ExitStack

import ml_dtypes
import concourse.bass as bass
import concourse.tile as tile
from concourse import bacc, mybir
from concourse._compat import with_exitstack
from concourse.bass_utils import run_bass_kernel_spmd

B = 4096
D = 256
R = 2 * B
N_CORES = 8
NBLK = 16            # global 512-row blocks
BLK = 512
PANEL = 10           # consecutive col blocks each core loads
ROWS = PANEL * BLK   # 5120
INV_T = 2.0
E2 = float(np.exp(2.0))

F32 = mybir.dt.float32
BF16 = mybir.dt.bfloat16
FP8 = mybir.dt.float8e4
DR = mybir.MatmulPerfMode.DoubleRow

# local col-block lists per exp group, r0 row-tiles (m 0..3) and r1 (m 4..7)
R0_GROUPS = [[0, 1, 2], [3, 4, 5], [6, 7, 8]]
R1_GROUPS = [[1, 2, 3], [4, 5, 6], [7, 8]]
# off-diagonal (row-tile-base, local col block) needing column sums
CS_BLOCKS = [(0, t) for t in range(1, 9)] + [(4, t) for t in range(2, 9)]
NCS = len(CS_BLOCKS)  # 15


@with_exitstack
def _loss_kernel(ctx: ExitStack, tc: "tile.TileContext", denacc_ap: bass.AP,
                 cs_ap: bass.AP, pos_ap: bass.AP, invn_ap: bass.AP,
                 x_ap: bass.AP, ident_ap: bass.AP):
    nc = tc.nc
    mult = mybir.AluOpType.mult
    Exp = mybir.ActivationFunctionType.Exp
    Ln = mybir.ActivationFunctionType.Ln

    xpool = ctx.enter_context(tc.tile_pool(name="x", bufs=1))
    spool = ctx.enter_context(tc.tile_pool(name="stats", bufs=1))
    zpool = ctx.enter_context(tc.tile_pool(name="z", bufs=4))
    jpool = ctx.enter_context(tc.tile_pool(name="junk", bufs=2))
    ztpool = ctx.enter_context(tc.tile_pool(name="zt", bufs=1))
    epool = ctx.enter_context(tc.tile_pool(name="esc", bufs=1))
    cpool = ctx.enter_context(tc.tile_pool(name="const", bufs=1))
    opool = ctx.enter_context(tc.tile_pool(name="outs", bufs=1))

    mpsum = ctx.enter_context(tc.tile_pool(name="mm", bufs=2, space="PSUM"))

    ident = cpool.tile([128, 128], F32, tag="ident")
    nc.sync.dma_start(ident[:], ident_ap[:])
    ones = cpool.tile([128, 1], BF16, tag="ones")
    nc.vector.memset(ones[:], 1.0)

    # ---- loads: 10 blocks of [512, 256] as [128, 4, 256] tiles ----------
    x = xpool.tile([128, 4 * PANEL, D], F32, tag="x")
    for b in range(PANEL):
        src = x_ap[b * BLK:(b + 1) * BLK, :].rearrange("(t p) d -> p t d", p=128)
        nc.gpsimd.dma_start(x[:, 4 * b:4 * b + 4, :], src)

    ssq = spool.tile([128, 4 * PANEL], F32, tag="ssq")
    lnv = spool.tile([128, 4 * PANEL], F32, tag="lnv")
    invn = spool.tile([128, 4 * PANEL], F32, tag="invn")
    zT = ztpool.tile([128, 2, ROWS], FP8, tag="zt")
    esc = epool.tile([128, 8, 9 * BLK], BF16, tag="esc")
    denacc = opool.tile([128, 24], F32, tag="denacc")
    pos = opool.tile([128, 8], F32, tag="pos")
    csb = opool.tile([1, NCS * BLK], F32, tag="csb")

    def emit_mm_group(gi):
        for m in range(8):
            groups = R0_GROUPS[gi] if m < 4 else R1_GROUPS[gi]
            width = BLK * len(groups)
            ptf = mpsum.tile([128, 3 * BLK], F32, tag="mm", name=f"pt{gi}_{m}")
            pt = ptf[:, :width]
            for i, t in enumerate(groups):
                nc.tensor.matmul(
                    pt[:, BLK * i:BLK * (i + 1)],
                    lhsT=zT[:, :, 128 * m:128 * (m + 1)],
                    rhs=zT[:, :, BLK * t:BLK * (t + 1)],
                    start=True, stop=True, perf_mode=DR,
                )
            t0 = groups[0]
            nc.scalar.activation(
                esc[:, m, BLK * t0:BLK * t0 + width], pt[:], Exp, scale=INV_T,
                accum_out=denacc[:, gi * 8 + m:gi * 8 + m + 1],
            )

    # ---- preamble per block pair + interleaved matmul groups ------------
    with tc.tile_pool(name="tp", bufs=2, space="PSUM") as tpsum:
        for g in range(5):
            bpair = (2 * g, 2 * g + 1)
            for b in bpair:
                for i in range(4 * b, 4 * b + 4):
                    junk = jpool.tile([128, D], F32, tag="junk", name=f"sq{i}")
                    nc.vector.scalar_tensor_tensor(
                        out=junk[:], in0=x[:, i, :], scalar=1.0,
                        in1=x[:, i, :], op0=mult, op1=mult,
                        accum_out=ssq[:, i:i + 1],
                    )
            sl = slice(8 * g, 8 * g + 8)
            nc.scalar.activation(lnv[:, sl], ssq[:, sl], Ln)
            nc.scalar.activation(invn[:, sl], lnv[:, sl], Exp, scale=-0.5)
            for b in bpair:
                for i in range(4 * b, 4 * b + 4):
                    z = zpool.tile([128, D], F32, tag="z", name=f"z{i}")
                    nc.vector.tensor_scalar(
                        out=z[:], in0=x[:, i, :],
                        scalar1=invn[:, i:i + 1], scalar2=None, op0=mult,
                    )
                    for k in range(2):
                        tp = tpsum.tile([128, 128], F32, tag="tp",
                                        name=f"tp{i}_{k}")
                        nc.tensor.transpose(tp[:], z[:, 128 * k:128 * (k + 1)],
                                            ident[:])
                        nc.vector.tensor_copy(zT[:, k, 128 * i:128 * (i + 1)],
                                              tp[:])
            if g == 1:
                emit_mm_group(0)
            elif g == 3:
                emit_mm_group(1)
    emit_mm_group(2)

    # ---- positives: raw row dots (host scales by invn) ------------------
    for m in range(8):
        junk = jpool.tile([128, D], F32, tag="junk", name=f"pp{m}")
        nc.vector.scalar_tensor_tensor(
            out=junk[:], in0=x[:, m, :], scalar=1.0,
            in1=x[:, 32 + m, :], op0=mult, op1=mult,
            accum_out=pos[:, m:m + 1],
        )

    # ---- column sums of the off-diagonal exp blocks ---------------------
    with tc.tile_pool(name="cs", bufs=2, space="PSUM") as cpsum:
        for bi, (mb, t) in enumerate(CS_BLOCKS):
            cs = cpsum.tile([1, BLK], F32, tag="cs", name=f"cs{bi}")
            for mm in range(4):
                nc.tensor.matmul(
                    cs[:],
                    lhsT=ones[:, 0:1],
                    rhs=esc[:, mb + mm, BLK * t:BLK * (t + 1)],
                    start=(mm == 0), stop=(mm == 3),
                )
            nc.vector.tensor_copy(csb[0:1, BLK * bi:BLK * (bi + 1)], cs[:])

    nc.sync.dma_start(denacc_ap[:], denacc[:])
    nc.sync.dma_start(pos_ap[:], pos[:])
    nc.sync.dma_start(invn_ap[:], invn[:])
    nc.sync.dma_start(cs_ap[:], csb[:])


_CACHE = {}


def _get_compiled():
    if "nc" not in _CACHE:
        nc = bacc.Bacc("TRN2", target_bir_lowering=False, debug=False)
        x_in = nc.dram_tensor("xin", [ROWS, D], F32, kind="ExternalInput")
        ident_t = nc.inline_tensor(np.eye(128, dtype=np.float32), name="ident")
        den_out = nc.dram_tensor("denacc", [128, 24], F32, kind="ExternalOutput")
        cs_out = nc.dram_tensor("colsum", [1, NCS * BLK], F32, kind="ExternalOutput")
        pos_out = nc.dram_tensor("pos", [128, 8], F32, kind="ExternalOutput")
        invn_out = nc.dram_tensor("invn", [128, 40], F32, kind="ExternalOutput")
        with tile.TileContext(nc) as tc:
            _loss_kernel(tc, den_out.ap(), cs_out.ap(), pos_out.ap(),
                         invn_out.ap(), x_in.ap(), ident_t.ap())
        nc.compile()
        _CACHE["nc"] = nc
    return _CACHE["nc"]


def make_in_maps(emb_i: np.ndarray, emb_j: np.ndarray):
    reps = np.concatenate(
        [np.asarray(emb_i, dtype=np.float32), np.asarray(emb_j, dtype=np.float32)],
        axis=0,
    )
    return [
        {"xin": np.ascontiguousarray(np.roll(reps, -c * 1024, axis=0)[:ROWS])}
        for c in range(N_CORES)
    ]


def run_spmd(emb_i, emb_j, **kwargs):
    nc = _get_compiled()
    in_maps = make_in_maps(emb_i, emb_j)
    return run_bass_kernel_spmd(nc, in_maps, core_ids=list(range(N_CORES)), **kwargs)


def assemble(results) -> np.ndarray:
    denom = np.zeros(R, dtype=np.float64)
    pos2 = np.zeros(R, dtype=np.float64)
    for c in range(N_CORES):
        r = results[c]
        da = r["denacc"].astype(np.float64)        # [128, 24]
        cs = r["colsum"].reshape(NCS, BLK).astype(np.float64)
        pr = r["pos"].astype(np.float64)           # [128, 8]
        iv = r["invn"].astype(np.float64)          # [128, 40]
        for m in range(8):
            s = da[:, m] + da[:, 8 + m] + da[:, 16 + m]
            blk = 2 * c if m < 4 else 2 * c + 1
            g0 = blk * BLK + (m % 4) * 128
            denom[g0:g0 + 128] += s
        for bi, (mb, t) in enumerate(CS_BLOCKS):
            j = (2 * c + t) % NBLK
            denom[j * BLK:(j + 1) * BLK] += cs[bi]
        for m in range(8):
            p2 = 2.0 * pr[:, m] * iv[:, m] * iv[:, 32 + m]
            g = 1024 * c + m * 128 + np.arange(128)
            pos2[g] = p2
            pos2[(g + B) % R] = p2
    denom -= E2
    loss = float(np.mean(np.log(denom) - pos2))
    return np.array(loss, dtype=np.float32)


def kernel(emb_i: np.ndarray, emb_j: np.ndarray) -> np.ndarray:
    res = run_spmd(emb_i, emb_j)
    return assemble(res.results)
